# revision 41
# baseline (speedup 1.0000x reference)
"""CapsNet forward on 8 Trainium2 NeuronCores — pure data parallel (64 imgs/core).

Self-contained: hardcodes all shapes (B=512 total). Host packs weights into
device layouts (pure layout transforms); the device does all FLOPs.

Per-core pipeline (B=64, chunks of 8 images, pconv weight passes over pairs):
  conv1: bf16 PE matmuls, K=81 im2col via shifted-replica DMA  -> feat (SBUF)
  pconv: 81 shifted pointwise matmuls (bf16), PSUM accumulation
  squash -> u in (s,k) layout, scattered to (gsub,s) i-major layout
  dynamic routing x3: PE for all contractions, DVE/ACT for softmax/elemwise
  decoder: 3 FC layers on PE in transposed orientation
Outputs per core: vlen [64,10], recon [64,784], vout [64,160] (f32).
"""
import numpy as np
import ml_dtypes

import concourse.bass as bass
import concourse.mybir as mybir
import concourse.tile as tile
from concourse import bacc
from concourse.bass_utils import run_bass_kernel_spmd
from concourse.masks import make_identity

F32 = mybir.dt.float32
BF16 = mybir.dt.bfloat16
AF = mybir.ActivationFunctionType
ALU = mybir.AluOpType
AX = mybir.AxisListType

B = 64
NCH = 8
NCHUNK = 8
EPS = 1e-8


# ---------------- i-permutation ----------------
# pconv out co = cap*8 + k; per cot=cap//16 psum partition = (cap%16)*8 + k
# i = (cot*16 + s)*36 + pos ; g = cot*36+pos ; gmaj = pos//4 ; gsub = cot*4+pos%4
# routing partition p1 = gsub*16 + s
def _iperm():
    p1 = np.arange(128)
    gsub, s = p1 // 16, p1 % 16
    cot, pm4 = gsub // 4, gsub % 4
    gmaj = np.arange(9)
    pos = gmaj[None, :] * 4 + pm4[:, None]
    return (cot[:, None] * 16 + s[:, None]) * 36 + pos  # [128, 9] -> orig i


def _bf(a):
    return np.ascontiguousarray(np.asarray(a, np.float32)).astype(ml_dtypes.bfloat16)


def _f32(a):
    return np.ascontiguousarray(np.asarray(a, np.float32))


def pack_weights(conv1_w, conv1_b, pconv_w, pconv_b, W,
                 fc1_w, fc1_b, fc2_w, fc2_b, fc3_w, fc3_b):
    m = {}
    m['w1p'] = _bf(np.asarray(conv1_w, np.float32).reshape(256, 81).T)
    m['b1'] = _f32(conv1_b)
    m['wpp'] = _bf(np.asarray(pconv_w, np.float32).transpose(2, 3, 1, 0)
                   .reshape(81, 256, 256))
    m['bp'] = _f32(pconv_b)
    iperm = _iperm()
    Wg = np.asarray(W, np.float32)[iperm]          # [128, 9, 10, 16, 8]
    m['Ws'] = _bf(Wg.transpose(0, 4, 1, 2, 3))     # [p1, k, gmaj, cls, d]
    m['Wd'] = _bf(Wg.transpose(3, 4, 1, 2, 0))     # [d, k, gmaj, cls, p1]
    f1 = np.zeros((256, 512), np.float32)
    f1[:160] = np.asarray(fc1_w, np.float32).T
    m['fc1p'] = _bf(f1)
    m['fb1'] = _f32(fc1_b)
    m['fc2p'] = _bf(np.asarray(fc2_w, np.float32).T)   # [512, 1024]
    m['fb2'] = _f32(fc2_b)
    f3 = np.zeros((1024, 896), np.float32)
    f3[:, :784] = np.asarray(fc3_w, np.float32).T
    m['fc3p'] = _bf(f3)
    fb3 = np.zeros(896, np.float32)
    fb3[:784] = np.asarray(fc3_b, np.float32)
    m['fb3'] = fb3
    o8 = np.zeros((128, 16), np.float32)
    for s in range(16):
        o8[s * 8:(s + 1) * 8, s] = 1.0
    m['ones8'] = _bf(o8)
    m['exp16'] = _bf(o8.T)
    return m


def build_nc():
    nc = bacc.Bacc(None, target_bir_lowering=False)
    io = {}
    def inp(name, shape, dt):
        io[name] = nc.declare_dram_parameter(name, list(shape), dt, isOutput=False)
    inp('xb', (B, 784), BF16)
    inp('mask', (B, 10), F32)
    inp('w1p', (81, 256), BF16)
    inp('b1', (256,), F32)
    inp('wpp', (81, 256, 256), BF16)
    inp('bp', (256,), F32)
    inp('Ws', (128, 8, 9, 10, 16), BF16)
    inp('Wd', (16, 8, 9, 10, 128), BF16)
    inp('fc1p', (256, 512), BF16)
    inp('fb1', (512,), F32)
    inp('fc2p', (512, 1024), BF16)
    inp('fb2', (1024,), F32)
    inp('fc3p', (1024, 896), BF16)
    inp('fb3', (896,), F32)
    inp('ones8', (128, 16), BF16)
    inp('exp16', (16, 128), BF16)
    io['vlen'] = nc.declare_dram_parameter('vlen', [B, 10], F32, isOutput=True)
    io['recon'] = nc.declare_dram_parameter('recon', [B, 784], F32, isOutput=True)
    io['vout'] = nc.declare_dram_parameter('vout', [B, 160], F32, isOutput=True)
    _build(nc, io)
    nc.compile()
    return nc


def _ap(t, offset_extra, dims):
    """Raw AP on a tile/handle with explicit [step, count] dims."""
    base = t[:] if not isinstance(t, bass.AP) else t
    return bass.AP(tensor=base.tensor, offset=base.offset + offset_extra, ap=dims)


def _build(nc, io):
    with nc.allow_low_precision('bf16 routing logits within tolerance'), \
         tile.TileContext(nc) as tc:
        with (
            tc.tile_pool(name='const', bufs=1) as C,
            tc.tile_pool(name='xrep', bufs=1) as XR,
            tc.tile_pool(name='feat', bufs=3) as FT,
            tc.tile_pool(name='wsl', bufs=6) as WS,
            tc.tile_pool(name='wd', bufs=1) as WD,
            tc.tile_pool(name='pers', bufs=1) as PR,
            tc.tile_pool(name='tmp', bufs=2) as TP,
            tc.tile_pool(name='ps_c1', bufs=1, space='PSUM') as PC1,
            tc.tile_pool(name='ps_pc', bufs=4, space='PSUM') as PPC,
            tc.tile_pool(name='ps_rt', bufs=2, space='PSUM') as PRT,
            tc.tile_pool(name='ps_sm', bufs=1, space='PSUM') as PSM,
        ):
            _body(nc, io, C, XR, FT, WS, WD, PR, TP, PC1, PPC, PRT, PSM)


def _body(nc, io, C, XR, FT, WS, WD, PR, TP, PC1, PPC, PRT, PSM):
    # ---------------- constants ----------------
    w1 = C.tile([81, 256], BF16)
    nc.sync.dma_start(out=w1, in_=io['w1p'][:, :])
    b1 = C.tile([128, 2], F32)
    nc.sync.dma_start(out=b1, in_=io['b1'].rearrange('(t p) -> p t', p=128))
    bp = C.tile([128, 2], F32)
    nc.sync.dma_start(out=bp, in_=io['bp'].rearrange('(t p) -> p t', p=128))
    Wst = C.tile([128, 8, 9, 10, 16], BF16)
    nc.sync.dma_start(out=Wst, in_=io['Ws'][:, :, :, :, :])
    mask_b = C.tile([B, 10], F32)
    nc.sync.dma_start(out=mask_b, in_=io['mask'][:, :])
    fc1 = C.tile([128, 2, 512], BF16)
    nc.sync.dma_start(out=fc1, in_=io['fc1p'].rearrange('(t p) m -> p t m', p=128))
    fc2 = C.tile([128, 4, 1024], BF16)
    nc.sync.dma_start(out=fc2, in_=io['fc2p'].rearrange('(t p) m -> p t m', p=128))
    fc3 = C.tile([128, 8, 896], BF16)
    nc.sync.dma_start(out=fc3, in_=io['fc3p'].rearrange('(t p) m -> p t m', p=128))
    fb1 = C.tile([128, 4], F32)
    nc.sync.dma_start(out=fb1, in_=io['fb1'].rearrange('(t p) -> p t', p=128))
    fb2 = C.tile([128, 8], F32)
    nc.sync.dma_start(out=fb2, in_=io['fb2'].rearrange('(t p) -> p t', p=128))
    fb3 = C.tile([128, 7], F32)
    nc.sync.dma_start(out=fb3, in_=io['fb3'].rearrange('(t p) -> p t', p=128))
    ones8 = C.tile([128, 16], BF16)   # ones8[(s,k), s'] = 1 iff s'==s
    nc.sync.dma_start(out=ones8, in_=io['ones8'][:, :])
    exp16 = C.tile([16, 128], BF16)   # exp16[s', (s,k)] = 1 iff s==s'
    nc.sync.dma_start(out=exp16, in_=io['exp16'][:, :])
    ident = C.tile([128, 128], F32)
    make_identity(nc, ident)
    eps_t = C.tile([128, 1], F32)
    nc.gpsimd.memset(eps_t, EPS)
    one_t = C.tile([128, 1], F32)
    nc.gpsimd.memset(one_t, 1.0)

    # ---------------- persistent activations ----------------
    u_kb = PR.tile([128, 9, 8, B], BF16)      # [(gsub,s), gmaj, k, b]
    u_sk = PR.tile([128, 2, 36, B], BF16)     # [(s,k), cot, pos, b]
    blog0 = PR.tile([128, 9, B, 10], BF16)    # [(gsub,s), gmaj, b, cls]
    c_T = PR.tile([128, 9, B, 10], BF16)      # softmax buf; doubles as blog1
    v_b = PR.tile([B, 10, 16], F32)
    s_b = PR.tile([B, 10, 16], F32)
    vT = PR.tile([16, 10, B], BF16)
    ns_b = PR.tile([B, 10], F32)
    scale_b = PR.tile([B, 10], F32)

    # ================= conv phase =================
    for pair in range(4):
        feats = []
        for ch in (pair * 2, pair * 2 + 1):
            feats.append(_conv1_chunk(nc, ch, io, w1, b1, XR, FT, PC1))
        psums = {}
        for ci in range(2):
            for cot in range(2):
                psums[(ci, cot)] = PPC.tile([128, NCH, 36], F32, tag='pc',
                                            name=f'pc_{ci}_{cot}')
        for kk in range(81):
            kh, kw = kk // 9, kk % 9
            for kt in range(2):
                wsl = WS.tile([128, 256], BF16, tag='wsl')
                nc.sync.dma_start(out=wsl,
                                  in_=io['wpp'][kk, kt * 128:(kt + 1) * 128, :])
                for ci in range(2):
                    f = feats[ci]
                    rhs = _ap(f, kt * (NCH * 400) + kh * 20 + kw,
                              [f[:].ap[0], [400, NCH], [40, 6], [2, 6]])
                    for cot in range(2):
                        nc.tensor.matmul(
                            psums[(ci, cot)][:, :, :],
                            wsl[:, cot * 128:(cot + 1) * 128],
                            rhs,
                            start=(kk == 0 and kt == 0),
                            stop=(kk == 80 and kt == 1))
        for ci in range(2):
            for cot in range(2):
                _squash_u(nc, pair * 2 + ci, cot, psums[(ci, cot)], bp,
                          ones8, exp16, u_sk, eps_t, one_t, TP, PSM)

    # scatter u_sk [(s,k), cot, pos, b] -> u_kb [(gsub,s), gmaj, k, b]
    FSs = u_sk[:].ap[0][0]
    FSk = u_kb[:].ap[0][0]
    for k in range(8):
        for gsub in range(8):
            cot, pm4 = gsub // 4, gsub % 4
            src = _ap(u_sk, k * FSs + cot * (36 * B) + pm4 * B,
                      [[8 * FSs, 16], [4 * B, 9], [1, B]])
            dst = _ap(u_kb, gsub * 16 * FSk + k * B,
                      [[FSk, 16], [8 * B, 9], [1, B]])
            nc.gpsimd.dma_start(out=dst, in_=src)
    # consolidate the 64 scatter DMAs into a single dependency for consumers
    nc.vector.tensor_copy(u_kb.rearrange('p g k b -> p (g k b)'),
                          u_kb.rearrange('p g k b -> p (g k b)'))

    # ================= routing =================
    for it in range(3):
        _route(nc, io, it, u_kb, Wst, blog0, c_T, v_b, s_b, vT,
               ns_b, scale_b, ident, eps_t, one_t, TP, WD, PRT, PSM)

    # vlen = sqrt(ns) * scale
    vlen = TP.tile([B, 10], F32, tag='vlen')
    nc.scalar.activation(vlen, ns_b, AF.Sqrt)
    nc.vector.tensor_mul(vlen, vlen, scale_b)
    nc.sync.dma_start(out=io['vlen'][:, :], in_=vlen)
    nc.sync.dma_start(out=io['vout'][:, :], in_=v_b.rearrange('b c d -> b (c d)'))

    # ================= decoder =================
    masked = TP.tile([B, 160], F32, tag='masked')
    nc.vector.scalar_tensor_tensor(
        out=masked, in0=v_b.rearrange('b c d -> b (c d)'), scalar=1.0,
        in1=_ap(mask_b, 0, [mask_b[:].ap[0], [1, 10], [0, 16]]),
        op0=ALU.mult, op1=ALU.mult)
    _decoder(nc, io, masked, fc1, fc2, fc3, fb1, fb2, fb3, ident, TP, PRT, PSM)


def _conv1_chunk(nc, ch, io, w1, b1, XR, FT, PC1):
    # xr[p=(dh,kw), f] = x_flat[ch*8*784 + f + dh*28 + kw], f in [0, 6040)
    xr = XR.tile([81, 6040], BF16, tag='xr')
    nc.sync.dma_start(
        out=xr,
        in_=bass.AP(tensor=io['xb'][:, :].tensor, offset=ch * NCH * 784,
                    ap=[[28, 9], [1, 9], [1, 6040]]))
    feat = FT.tile([128, 2, NCH, 400], BF16, tag='feat')
    for b in range(NCH):
        rhs = _ap(xr, b * 784, [xr[:].ap[0], [28, 20], [1, 20]])
        for cit in range(2):
            ps = PC1.tile([128, 400], F32, tag='c1')
            nc.tensor.matmul(ps, w1[:, cit * 128:(cit + 1) * 128], rhs,
                             start=True, stop=True)
            nc.scalar.activation(feat[:, cit, b, :], ps, AF.Relu,
                                 bias=b1[:, cit:cit + 1], scale=1.0)
    return feat


def _squash_u(nc, ch, cot, pps, bp, ones8, exp16, u_sk, eps_t, one_t, TP, PSM):
    """pps [128=(s,k), NCH, 36] f32 psum -> u_sk[:, cot, :, ch-slice] bf16."""
    p_sb = TP.tile([128, NCH, 36], F32, tag='p_sb')
    nc.scalar.activation(p_sb, pps, AF.Identity, bias=bp[:, cot:cot + 1])
    p2 = TP.tile([128, NCH * 36], BF16, tag='p2')
    nc.vector.scalar_tensor_tensor(
        out=p2, in0=p_sb.rearrange('p b q -> p (b q)'), scalar=1.0,
        in1=p_sb.rearrange('p b q -> p (b q)'), op0=ALU.mult, op1=ALU.mult)
    ns = PSM.tile([16, NCH * 36], F32, tag='sm')
    nc.tensor.matmul(ns, ones8, p2, start=True, stop=True)
    t = TP.tile([16, NCH * 36], F32, tag='sq_t')
    u = TP.tile([16, NCH * 36], F32, tag='sq_u')
    nc.scalar.activation(t, ns, AF.Sqrt)          # sqrt(ns)
    nc.scalar.activation(t, t, AF.Identity, bias=eps_t[0:16, 0:1])
    nc.scalar.activation(u, ns, AF.Identity, bias=one_t[0:16, 0:1])
    nc.vector.tensor_mul(t, t, u)                  # (1+ns)(sqrt+eps)
    nc.vector.reciprocal(t, t)
    sc = TP.tile([16, NCH * 36], BF16, tag='sq_sc')
    nc.vector.scalar_tensor_tensor(out=sc, in0=ns, scalar=1.0, in1=t,
                                   op0=ALU.mult, op1=ALU.mult)  # ns * recip
    scb = PSM.tile([128, NCH * 36], F32, tag='sm')
    nc.tensor.matmul(scb, exp16, sc, start=True, stop=True)
    # u = p * scale_bc -> out AP [cot, pos, b] (free transpose)
    out = _ap(u_sk, cot * (36 * B) + ch * NCH,
              [u_sk[:].ap[0], [1, NCH], [B, 36]])
    nc.vector.tensor_mul(out, p_sb, scb.rearrange('p (b q) -> p b q', b=NCH))


def _route(nc, io, it, u_kb, Wst, blog0, c_T, v_b, s_b, vT,
           ns_b, scale_b, ident, eps_t, one_t, TP, WD, PRT, PSM):
    FSc = c_T[:].ap[0][0]
    FSu = u_kb[:].ap[0][0]
    if it == 0:
        sps = PSM.tile([B, 160], F32, tag='sm')
        n = 0
        for k in range(8):
            for gmaj in range(9):
                nc.tensor.matmul(sps, u_kb[:, gmaj, k, :],
                                 Wst[:, k, gmaj, :, :].rearrange('p c d -> p (c d)'),
                                 start=(n == 0), stop=(n == 71))
                n += 1
        nc.scalar.activation(s_b.rearrange('b c d -> b (c d)'), sps,
                             AF.Copy, scale=0.1)
    else:
        # softmax over cls (free innermost); it==1 reads blog0, it==2 reads
        # c_T itself (which holds blog1 = blog0 + agree1), exp in place.
        src = blog0 if it == 1 else c_T
        nc.scalar.activation(c_T.rearrange('p g b c -> p (g b c)'),
                             src.rearrange('p g b c -> p (g b c)'), AF.Exp)
        z = TP.tile([128, 9 * B], F32, tag='z')
        nc.vector.tensor_reduce(z.rearrange('p (g b) -> p g b', g=9),
                                c_T[:, :, :, :], axis=AX.X, op=ALU.add)
        nc.vector.reciprocal(z, z)
        nc.vector.scalar_tensor_tensor(
            out=c_T[:, :, :, :], in0=c_T[:, :, :, :], scalar=1.0,
            in1=_ap(z, 0, [z[:].ap[0], [B, 9], [1, B], [0, 10]]),
            op0=ALU.mult, op1=ALU.mult)
        # s matmuls via c-weighted u
        for cls in range(10):
            spsT = PSM.tile([16, B], F32, tag='sm')
            n = 0
            for k in range(8):
                uh = TP.tile([128, 9, B], BF16, tag='uh')
                nc.vector.scalar_tensor_tensor(
                    out=uh, in0=_ap(c_T, cls, [[FSc, 128], [B * 10, 9], [10, B]]),
                    scalar=1.0, in1=u_kb[:, :, k, :], op0=ALU.mult, op1=ALU.mult)
                for gmaj in range(9):
                    nc.tensor.matmul(spsT, Wst[:, k, gmaj, cls, :],
                                     uh[:, gmaj, :],
                                     start=(n == 0), stop=(n == 71))
                    n += 1
            sdT = TP.tile([16, B], F32, tag='sdT')
            nc.scalar.copy(sdT, spsT)
            stp = PSM.tile([B, 16], F32, tag='sm')
            nc.tensor.transpose(stp, sdT, ident[0:16, 0:16])
            nc.scalar.copy(s_b[:, cls, :], stp)
    _squash_s(nc, it, s_b, v_b, ns_b, scale_b, eps_t, one_t, TP)
    if it < 2:
        # vT[d, cls, b]
        for cls in range(10):
            vtp = PSM.tile([16, B], F32, tag='sm')
            nc.tensor.transpose(vtp, v_b[:, cls, :], ident[0:B, 0:B])
            nc.scalar.copy(vT[:, cls, :], vtp)
        # agree (round 1 writes into c_T, which then becomes blog1)
        dest = blog0 if it == 0 else c_T
        FSd = dest[:].ap[0][0]
        for gmaj in range(9):
            wd = WD.tile([16, 8, 10, 128], BF16, tag='wd')
            nc.sync.dma_start(out=wd, in_=io['Wd'][:, :, gmaj, :, :])
            for cls in range(10):
                pp = PRT.tile([128, 8, B], F32, tag='pp')
                for k in range(8):
                    nc.tensor.matmul(pp[:, k, :], wd[:, k, cls, :],
                                     vT[:, cls, :], start=True, stop=True)
                q = TP.tile([128, B, 8], F32, tag='q')
                nc.vector.tensor_mul(
                    _ap(q, 0, [q[:].ap[0], [1, 8], [8, B]]),
                    pp[:, :, :], u_kb[:, gmaj, :, :])
                nc.vector.tensor_reduce(
                    _ap(dest, gmaj * (B * 10) + cls, [[FSd, 128], [10, B]]),
                    q[:, :, :], axis=AX.X, op=ALU.add)
        if it == 1:
            nc.vector.tensor_tensor(
                out=c_T.rearrange('p g b c -> p (g b c)'),
                in0=c_T.rearrange('p g b c -> p (g b c)'),
                in1=blog0.rearrange('p g b c -> p (g b c)'), op=ALU.add)


def _squash_s(nc, it, s_b, v_b, ns_b, scale_b, eps_t, one_t, TP):
    s2 = TP.tile([B, 160], F32, tag='s2')
    sf = s_b.rearrange('b c d -> b (c d)')
    nc.vector.scalar_tensor_tensor(out=s2, in0=sf, scalar=1.0, in1=sf,
                                   op0=ALU.mult, op1=ALU.mult)
    nc.vector.tensor_reduce(ns_b, s2.rearrange('b (c d) -> b c d', c=10),
                            axis=AX.X, op=ALU.add)
    t = TP.tile([B, 10], F32, tag='sq_t2')
    u = TP.tile([B, 10], F32, tag='sq_u2')
    nc.scalar.activation(t, ns_b, AF.Sqrt)
    nc.scalar.activation(t, t, AF.Identity, bias=eps_t[0:B, 0:1])
    nc.scalar.activation(u, ns_b, AF.Identity, bias=one_t[0:B, 0:1])
    nc.vector.tensor_mul(t, t, u)
    nc.vector.reciprocal(t, t)
    nc.vector.tensor_mul(scale_b, ns_b, t)
    nc.vector.scalar_tensor_tensor(
        out=v_b.rearrange('b c d -> b (c d)'), in0=sf, scalar=1.0,
        in1=_ap(scale_b, 0, [scale_b[:].ap[0], [1, 10], [0, 16]]),
        op0=ALU.mult, op1=ALU.mult)


def _decoder(nc, io, masked, fc1, fc2, fc3, fb1, fb2, fb3, ident, TP, PRT, PSM):
    # masked_T [160->256 pad, B]
    mT = TP.tile([128, 2, B], BF16, tag='mT')
    nc.vector.memset(mT, 0.0)
    t1 = PSM.tile([128, B], F32, tag='sm')
    nc.tensor.transpose(t1, masked[:, 0:128], ident[0:B, 0:B])
    nc.scalar.copy(mT[:, 0, :], t1)
    t2 = PSM.tile([32, B], F32, tag='sm')
    nc.tensor.transpose(t2, masked[:, 128:160], ident[0:B, 0:B])
    nc.scalar.copy(mT[0:32, 1, :], t2)
    # fc1: h1T [128, 4, B]
    h1 = TP.tile([128, 4, B], BF16, tag='h1')
    for mt in range(4):
        hp = PSM.tile([128, B], F32, tag='sm')
        for kt in range(2):
            nc.tensor.matmul(hp, fc1[:, kt, mt * 128:(mt + 1) * 128], mT[:, kt, :],
                             start=(kt == 0), stop=(kt == 1))
        nc.scalar.activation(h1[:, mt, :], hp, AF.Relu, bias=fb1[:, mt:mt + 1])
    # fc2: h2T [128, 8, B]
    h2 = TP.tile([128, 8, B], BF16, tag='h2')
    for mt in range(8):
        hp = PSM.tile([128, B], F32, tag='sm')
        for kt in range(4):
            nc.tensor.matmul(hp, fc2[:, kt, mt * 128:(mt + 1) * 128], h1[:, kt, :],
                             start=(kt == 0), stop=(kt == 3))
        nc.scalar.activation(h2[:, mt, :], hp, AF.Relu, bias=fb2[:, mt:mt + 1])
    # fc3 + sigmoid: reconS [128, 7, B] f32
    reconS = TP.tile([128, 7, B], F32, tag='reconS')
    for mt in range(7):
        hp = PSM.tile([128, B], F32, tag='sm')
        for kt in range(8):
            nc.tensor.matmul(hp, fc3[:, kt, mt * 128:(mt + 1) * 128], h2[:, kt, :],
                             start=(kt == 0), stop=(kt == 7))
        nc.scalar.activation(reconS[:, mt, :], hp, AF.Sigmoid,
                             bias=fb3[:, mt:mt + 1])
    # transpose back to [B, 784] and store
    rb = TP.tile([B, 128], F32, tag='rb')
    for mt in range(7):
        rp = PSM.tile([B, 128], F32, tag='sm')
        nc.tensor.transpose(rp, reconS[:, mt, :], ident[:, :])
        ncols = 128 if mt < 6 else 16
        nc.scalar.copy(rb[:, 0:ncols], rp[:, 0:ncols])
        nc.sync.dma_start(out=io['recon'][:, mt * 128:mt * 128 + ncols],
                          in_=rb[:, 0:ncols])


_NC_CACHE = None


def kernel(**inputs):
    global _NC_CACHE
    x = np.asarray(inputs['x'], np.float32)          # [512, 1, 28, 28]
    labels = np.asarray(inputs['labels'])
    BT = x.shape[0]
    ncores = 8
    bl = BT // ncores
    assert bl == B
    wm = pack_weights(
        inputs['conv1_w'], inputs['conv1_b'], inputs['pconv_w'], inputs['pconv_b'],
        inputs['W'], inputs['fc1_w'], inputs['fc1_b'], inputs['fc2_w'],
        inputs['fc2_b'], inputs['fc3_w'], inputs['fc3_b'])
    mask = np.zeros((BT, 10), np.float32)
    mask[np.arange(BT), np.asarray(labels, np.int64)] = 1.0
    in_maps = []
    for c in range(ncores):
        sl = slice(c * bl, (c + 1) * bl)
        im = dict(wm)
        im['xb'] = x[sl].reshape(bl, 784).astype(ml_dtypes.bfloat16)
        im['mask'] = mask[sl]
        in_maps.append(im)
    if _NC_CACHE is None:
        _NC_CACHE = build_nc()
    res = run_bass_kernel_spmd(_NC_CACHE, in_maps, list(range(ncores))).results
    vlen = np.concatenate([r['vlen'] for r in res], axis=0)
    recon = np.concatenate([r['recon'] for r in res], axis=0)
    v = np.concatenate([r['vout'] for r in res], axis=0).reshape(BT, 10, 16)
    return (vlen, recon, v)
